# revision 117
# baseline (speedup 1.0000x reference)
"""Trainium2 Bass kernel for a 4-head attention layer with post-softmax
affine blend (attn = 0.5*softmax(qk/sqrt(dh)) + 0.5), distributed over 8
NeuronCores.

Reference computation (B=2, S=4096, D=128, H=4, Dh=32):
    k = einsum('ihd,bpd->biph', W_K, x)
    q = einsum('ihd,bpd->biph', W_Q, x)
    v = einsum('ihd,bpd->biph', W_V, x)
    scores = einsum('biph,biqh->biqp', k, q) / sqrt(32)
    attn   = softmax(scores, -1) * 0.5 + 0.5
    z      = einsum('biph,biqp->biqh', v, attn)
    out    = einsum('df,bpf->bpd', W_O, z_flat)

Sharding: 8 cores = (batch b in {0,1}) x (query chunk qc in 4 x 1024).
Each core computes all 4 heads for its 1024 queries against all 4096
keys and emits the disjoint output slice out[b, qc*1024:(qc+1)*1024, :].

Per-core pipeline (everything stays on-chip), restructured for engine
balance: the kernel iterates over 128 "prounds" (qh in 2 q-halves of
512, kb in 32 key-blocks of 128, p in 2 head-pairs). Per pround:
  - scores^T tile [128 keys x 1024 (2 heads x 512 q)] via 2 row-banded
    K=32 matmuls into a 2-bank PSUM tile (2-buffer rotation so the PE
    can run one pround ahead of the exp stage).
  - exp alternates engines by head-pair parity: ScalarE activation
    (p=0) and a custom Schraudolph exp2 DVE op (p=1), so the two
    elementwise engines each carry half the 16.8M-exp/core load.
  - z^T accumulates in PSUM over key-blocks with col-packed M=64
    matmuls whose stationary operand is [v_i | ones | zero-pad]; the
    softmax denominator accumulates in the same PSUM tile for free.
  - The uniform 0.5*sum_k(v) blend term is a host-computed per-batch
    constant c, partition-broadcast once on GpSimd and added to the
    output tiles on GpSimd; softmax normalization uses a fast DVE
    reciprocal + GpSimd partition-broadcast (no DRAM bounce).
"""

import math

import numpy as np
import ml_dtypes

BF16 = ml_dtypes.bfloat16

B, S, D, H, DH = 2, 4096, 128, 4, 32
QCHUNK = 1024  # queries per core
NCORES = 8
NKB = S // 128  # 32 key blocks
# exp(s) is computed as exp((s * 2^15 * log2(e)) * ln(2) / 2^15); the big
# pre-scale is folded into W_Q so a bit-trick exp2 on VectorE can share the
# same score tensor with ScalarE's table exp.
PRESCALE = (2.0**15) * math.log2(math.e) / math.sqrt(DH)
ACT_SCALE = math.log(2.0) / (2.0**15)

# Schraudolph exp2 constant: sigma balances the multiplicative error of the
# linear-mantissa approximation; folded into the int16 bf16-bit construction.
EXP2_SIGMA = 0.02979

_PROGRAM = None


def _register_exp2():
    """Register (once) a fused y = x*C0 + C1 custom DVE op whose int16
    output, reinterpreted as bf16, is 2^(x/2^15) a la Schraudolph."""
    from concourse import dve_ops
    from concourse.dve_spec import Spec, Src0, C0, C1, lower, _has_src1
    from concourse.dve_uop import DveOpSpec

    name = "EXP2_SCHRAU_ANT"
    for o in dve_ops.OPS:
        if o.name == name:
            return o
    spec = Spec(body=Src0 * C0 + C1,
                reference=lambda in0, in1, c0, c1, c2: in0 * c0 + c1)
    opcode = dve_ops._CUSTOM_DVE_ROW_BASE + len(dve_ops.OPS)
    shas = {}
    for ver in ("v3", "v4"):
        s = DveOpSpec(name=name, opcode=opcode, uops=lower(spec, ver=ver),
                      rd1_en=_has_src1(spec))
        shas[ver] = s.sha(ver)
    op = dve_ops.DveOp(name, spec, subdim=False, uops_sha=shas)
    dve_ops.OPS.append(op)
    dve_ops.CUSTOM_DVE_SPECS[name] = spec
    dve_ops._SUB_OPCODE_FOR_NAME[name] = opcode
    return op


def _build_program():
    import concourse.bass as bass
    import concourse.mybir as mybir
    import concourse.tile as tile
    from concourse import bacc
    from contextlib import ExitStack

    import dataclasses

    f32 = mybir.dt.float32
    bf16 = mybir.dt.bfloat16
    AF = mybir.ActivationFunctionType
    exp2_op = _register_exp2()

    def i16_alias(ap):
        h = dataclasses.replace(ap.tensor, dtype=mybir.dt.int16)
        return bass.AP(tensor=h, offset=ap.offset, ap=[list(d) for d in ap.ap])

    nc = bacc.Bacc(None, target_bir_lowering=False)

    xkT = nc.dram_tensor("xkT", [D, S], bf16, kind="ExternalInput")
    xqT = nc.dram_tensor("xqT", [D, QCHUNK], bf16, kind="ExternalInput")
    wqT = nc.dram_tensor("wqT", [D, H * DH], bf16, kind="ExternalInput")
    wkT = nc.dram_tensor("wkT", [D, H * DH], bf16, kind="ExternalInput")
    wvT = nc.dram_tensor("wvT", [D, H * DH], bf16, kind="ExternalInput")
    woT = nc.dram_tensor("woT", [2, 128, D], bf16, kind="ExternalInput")
    cvec = nc.dram_tensor("cvec", [1, D], f32, kind="ExternalInput")
    out = nc.dram_tensor("out", [QCHUNK, D], f32, kind="ExternalOutput")

    with tile.TileContext(nc) as tc, ExitStack() as ctx:
        const = ctx.enter_context(tc.tile_pool(name="const", bufs=1))
        work = ctx.enter_context(tc.tile_pool(name="work", bufs=1))

        # ---- constants / persistent SBUF tensors ----
        # input DMAs spread across engine DGE queues so the descriptor
        # pushes don't serialize on the Sync engine
        w_sb = {}
        for eng, (name, dram) in zip(
            (nc.scalar, nc.gpsimd, nc.sync), (("wq", wqT), ("wk", wkT), ("wv", wvT))
        ):
            t = const.tile([128, 128], bf16, tag=f"w_{name}", name=f"w_{name}")
            eng.dma_start(out=t, in_=dram[:, :])
            w_sb[name] = t
        wo_sb = const.tile([128, 2, 128], bf16, tag="wo_sb")
        for p in range(2):
            nc.gpsimd.dma_start(out=wo_sb[:, p, :], in_=woT[p, :, :])
        # c broadcast to all partitions straight from DRAM (partition-stride-0
        # read), for the final output add on DVE
        c_rep = const.tile([128, D], f32, tag="c_rep")
        _c0 = cvec[0, :]
        c_bcast = bass.AP(tensor=_c0.tensor, offset=_c0.offset, ap=[[0, 128], [1, 128]])
        nc.gpsimd.dma_start(out=c_rep, in_=c_bcast)
        zrow = const.tile([1, 512], bf16, tag="zrow")
        nc.vector.memset(zrow, 0.0)

        xq_sb = const.tile([128, QCHUNK], bf16, tag="xq_sb")
        nc.scalar.dma_start(out=xq_sb, in_=xqT[:, :])
        xk_sb = const.tile([128, S], bf16, tag="xk_sb")
        kT_sb = const.tile([128, S], bf16, tag="kT_sb")
        qT_sb = const.tile([128, QCHUNK], bf16, tag="qT_sb")
        # v_sb[key, kb, head, 0:32]=v, [...,32]=1.0 (denominator ones row);
        # M=33 stationary — PSUM rows 33:63 of each col strip are never
        # written or read, so no zero padding (and no big memset) is needed.
        v_sb = const.tile([128, NKB, H, 33], bf16, tag="v_sb")
        nc.gpsimd.memset(v_sb[:, :, :, 32], 1.0)

        # ---- PSUM pools: 3 dual-bank score tiles (6 banks, 3-pround-deep
        # rotation) + 2 z banks. The projection matmuls borrow rotation
        # slots (pk in cols 0:512, the four pv's in cols 512:1024), so no
        # separate projection banks are needed.
        dram_pool = ctx.enter_context(
            tc.tile_pool(name="dram_pool", bufs=1, space="DRAM")
        )
        zden_ps = ctx.enter_context(tc.tile_pool(name="zden_ps", bufs=1, space="PSUM"))
        round_ctx = ExitStack()
        st_ps = round_ctx.enter_context(tc.tile_pool(name="st_ps", bufs=1, space="PSUM"))
        exp_pool = round_ctx.enter_context(tc.tile_pool(name="exp_pool", bufs=3))
        st_rr = [0]

        def grab_slot(name):
            tag = f"st_{st_rr[0] % 3}"
            st_rr[0] += 1
            return st_ps.tile([128, 1024], f32, tag=tag, name=name)

        # reciprocal staging on partition 0: 4 segments (one per p,j)
        rec = work.tile([1, 4, 512], f32, tag="rec", name="rec")
        # SBUF copies of the qh=0 z accumulators: ScalarE copies these out
        # at the qh boundary so qh=1's z matmuls get the PSUM banks back
        # immediately instead of waiting on the serial reciprocal chain
        zc = [work.tile([128, 512], f32, tag=f"zc_{p}", name=f"zc_{p}")
              for p in range(2)]
        rec_dram = [
            [dram_pool.tile([1, 512], f32, tag=f"rd_{p}_{j}_{qh}",
                            name=f"rd_{p}_{j}_{qh}") for j in range(2) for qh in range(2)]
            for p in range(2)
        ]
        rep = [work.tile([128, QCHUNK], f32, tag=f"rep_{p}", name=f"rep_{p}") for p in range(2)]
        zT_sb = [work.tile([128, QCHUNK], bf16, tag=f"zT_{p}", name=f"zT_{p}") for p in range(2)]

        # z/denominator accumulators: [pair] -> [128, 512] for the current
        # q-half; rows 0:32 z of head 2p, row 32 its denom, rows 64:96 z of
        # head 2p+1, row 96 its denom. qh1 reuses qh0's banks (bufs=1 tags)
        # once qh0's normalization has read them.
        z_cur = [None, None]

        def emit_proj_chunk(c8):
            sl = slice(c8 * 512, (c8 + 1) * 512)
            nc.sync.dma_start(out=xk_sb[:, sl], in_=xkT[:, sl])
            slot = grab_slot("proj")
            pk = slot[:, 0:512]
            nc.tensor.matmul(pk, w_sb["wk"], xk_sb[:, sl], start=True, stop=True)
            nc.scalar.copy(out=kT_sb[:, sl], in_=pk)
            for j in range(4):  # 128-col key blocks inside the chunk
                kb = c8 * 4 + j
                ksl = slice(kb * 128, (kb + 1) * 128)
                pv = slot[:, 512 + 128 * j : 640 + 128 * j]
                nc.tensor.matmul(pv, xk_sb[:, ksl], w_sb["wv"], start=True, stop=True)
            # one batched scatter of all four key blocks into the
            # [kb, head, 33] aug layout (4x fewer DVE copy overheads)
            nc.vector.tensor_copy(
                out=v_sb[:, 4 * c8 : 4 * c8 + 4, :, 0:32],
                in_=slot[:, 512:1024].rearrange("p (kb i h) -> p kb i h", kb=4, i=H),
            )

        def emit_qproj():
            slot = grab_slot("qproj")
            for qh in range(2):
                sl = slice(qh * 512, (qh + 1) * 512)
                pq = slot[:, qh * 512 : (qh + 1) * 512]
                nc.tensor.matmul(pq, w_sb["wq"], xq_sb[:, sl], start=True, stop=True)
                nc.vector.tensor_copy(out=qT_sb[:, sl], in_=pq)

        def emit_scores(idx, qh, kb, p):
            qsl = slice(qh * 512, (qh + 1) * 512)
            ksl = slice(kb * 128, (kb + 1) * 128)
            st = grab_slot("st")
            for j in range(2):
                i = 2 * p + j
                nc.tensor.matmul(
                    st[:, j * 512 : (j + 1) * 512],
                    kT_sb[32 * i : 32 * (i + 1), ksl],
                    qT_sb[32 * i : 32 * (i + 1), qsl],
                    start=True,
                    stop=True,
                    tile_position=(32 * i, 0),
                )
            return st

        def emit_exp(st, p):
            e = exp_pool.tile([128, 1024], bf16, tag=f"ex_{p}", name=f"ex_{p}")
            if p == 0:
                nc.scalar.activation(out=e, in_=st, func=AF.Exp, scale=ACT_SCALE)
            else:
                nc.vector._custom_dve(
                    exp2_op, out=i16_alias(e[:, :]), in0=st[:, :],
                    s0=1.0 / 256.0, s1=(127.0 - EXP2_SIGMA) * 128.0,
                )
            return e

        def emit_zinit(qh, p):
            z_cur[p] = zden_ps.tile(
                [128, 512], f32, tag=f"z_{p}", name=f"z_{p}"
            )
            nc.tensor.matmul(
                z_cur[p], zrow[:, 0:128], zrow, start=True, stop=False,
                skip_group_check=True,
            )

        def emit_z(qh, kb, p, e):
            if kb == 0:
                emit_zinit(qh, p)
            for j in range(2):
                nc.tensor.matmul(
                    z_cur[p][64 * j : 64 * j + 33, :],
                    v_sb[:, kb, 2 * p + j, :],
                    e[:, j * 512 : (j + 1) * 512],
                    start=False,
                    stop=(kb == NKB - 1),
                    tile_position=(0, 64 * j),
                    skip_group_check=True,
                )

        def emit_epilogue(qh):
            """Emit denominator-reciprocal chain; return deferred normalize-
            multiply closures (emitted later so the DRAM broadcast latency
            hides behind compute instead of head-of-line-blocking the DVE)."""
            # per-qh normalization: fast reciprocal of the denominator rows
            # on DVE, partition-broadcast on GpSimd, then the normalizing
            # multiply (PSUM z rows x broadcast recip) on DVE.
            qsl = slice(qh * 512, (qh + 1) * 512)
            if qh == 0:
                for p in range(2):
                    nc.scalar.copy(out=zc[p][0:33, :], in_=z_cur[p][0:33, :])
                    nc.scalar.copy(out=zc[p][64:97, :], in_=z_cur[p][64:97, :])
                srcs = zc
            else:
                srcs = [z_cur[0], z_cur[1]]
            recips, muls = [], []
            for p in range(2):
                for j in range(2):
                    def recip_unit(p=p, j=j, src_t=srcs[p]):
                        r = 64 * j + 32
                        seg = 2 * j + p
                        nc.vector.reciprocal(
                            out=rec[0:1, seg, :], in_=src_t[r : r + 1, :]
                        )
                        rd = rec_dram[p][2 * j + qh]
                        nc.sync.dma_start(out=rd, in_=rec[0:1, seg, :])
                        src = rd[0, :]
                        bcast = bass.AP(
                            tensor=src.tensor, offset=src.offset,
                            ap=[[0, 32], [1, 512]],
                        )
                        nc.gpsimd.dma_start(
                            out=rep[p][64 * j : 64 * j + 32, qsl], in_=bcast
                        )

                    def mul_unit(p=p, j=j, src_t=srcs[p]):
                        rsl = slice(64 * j, 64 * j + 32)
                        nc.vector.tensor_mul(
                            zT_sb[p][rsl, qsl], src_t[rsl, :], rep[p][rsl, qsl]
                        )
                    recips.append(recip_unit)
                    muls.append(mul_unit)
            return recips, muls

        # ---- main pipeline ----
        emit_qproj()
        emit_proj_chunk(0)
        # Round granularity (qh, kb): all four heads' score matmuls issue as
        # ONE 4-way row-banded quad — a quad occupies the same PE slot as a
        # pair (concurrent row bands), cutting PE groups per round from 4
        # (2 score pairs + 2 z pairs) to 3 (1 quad + 2 z pairs).
        rounds = [(qh, kb) for qh in range(2) for kb in range(NKB)]
        pending = None  # (qh, kb, [ex_p0, ex_p1]) for the z stage, one round behind
        def_recips, def_muls = [], []  # deferred epilogue(0) units
        for qh, kb in rounds:
            if qh == 0 and kb % 4 == 0 and kb < 28:
                emit_proj_chunk(kb // 4 + 1)  # prefetch one chunk ahead
            sts = [emit_scores(0, qh, kb, p) for p in range(2)]
            es = [emit_exp(sts[p], p) for p in range(2)]
            if qh == 1:
                # epilogue(0) work spread into qh=1: reciprocal+broadcast
                # issues early (kb 2..8), normalize-muls ~13us later so the
                # DRAM broadcast round-trip hides under compute
                if def_recips and kb % 2 == 0 and kb >= 2:
                    def_recips.pop(0)()
                if def_muls and kb % 4 == 0 and kb >= 16:
                    def_muls.pop(0)()
            if pending is not None:
                pqh, pkb, pes = pending
                for p in range(2):
                    emit_z(pqh, pkb, p, pes[p])
            pending = (qh, kb, es)
            if kb == NKB - 1:
                for p in range(2):
                    emit_z(qh, kb, p, es[p])
                pending = None
                recips, muls = emit_epilogue(qh)
                if qh == 0:
                    def_recips, def_muls = recips, muls
                else:
                    for u in recips + muls:
                        u()

        round_ctx.close()

        # ---- final projection + blend constant ----
        with tc.tile_pool(name="u_ps", bufs=2, space="PSUM") as u_ps, tc.tile_pool(
            name="out_pool", bufs=2
        ) as out_pool:
            for qb in range(QCHUNK // 128):
                bsl = slice(qb * 128, (qb + 1) * 128)
                ue = u_ps.tile([128, 128], f32, tag="ue")
                uo = u_ps.tile([128, 128], f32, tag="uo")
                # heads 0,2 -> ue (lhsT partitions 0:32); heads 1,3 -> uo (64:96)
                nc.tensor.matmul(
                    ue, zT_sb[0][0:32, bsl], wo_sb[0:32, 0, :], start=True, stop=False,
                    skip_group_check=True, tile_position=(0, 0),
                )
                nc.tensor.matmul(
                    ue, zT_sb[1][0:32, bsl], wo_sb[0:32, 1, :], start=False,
                    stop=True, skip_group_check=True, tile_position=(0, 0),
                )
                nc.tensor.matmul(
                    uo, zT_sb[0][64:96, bsl], wo_sb[64:96, 0, :], start=True,
                    stop=False, skip_group_check=True, tile_position=(64, 0),
                )
                nc.tensor.matmul(
                    uo, zT_sb[1][64:96, bsl], wo_sb[64:96, 1, :], start=False,
                    stop=True, skip_group_check=True, tile_position=(64, 0),
                )
                ob = out_pool.tile([128, 128], f32, tag="ob")
                nc.scalar.copy(out=ob, in_=ue)
                nc.vector.tensor_add(ob, ob, uo)
                nc.vector.tensor_add(ob, ob, c_rep)
                nc.sync.dma_start(out=out[bsl, :], in_=ob)

    nc.compile()
    return nc


def _get_program():
    global _PROGRAM
    if _PROGRAM is None:
        _PROGRAM = _build_program()
    return _PROGRAM


def make_in_maps(x, W_K, W_Q, W_V, W_O):
    x = np.asarray(x, np.float32)
    W_K = np.asarray(W_K, np.float32)
    W_Q = np.asarray(W_Q, np.float32)
    W_V = np.asarray(W_V, np.float32)
    W_O = np.asarray(W_O, np.float32)

    wqT = np.ascontiguousarray((W_Q.transpose(2, 0, 1).reshape(D, H * DH)) * PRESCALE)
    wkT = np.ascontiguousarray(W_K.transpose(2, 0, 1).reshape(D, H * DH))
    wvT = np.ascontiguousarray(W_V.transpose(2, 0, 1).reshape(D, H * DH))
    woT_flat = 0.5 * W_O.T  # [f, d']
    woT = np.zeros((2, 128, D), np.float32)
    for p in range(2):
        woT[p, 0:32] = woT_flat[(2 * p) * 32 : (2 * p) * 32 + 32]
        woT[p, 64:96] = woT_flat[(2 * p + 1) * 32 : (2 * p + 1) * 32 + 32]

    in_maps = []
    for core in range(NCORES):
        b, qc = divmod(core, 4)
        xb = x[b]
        xkT_b = np.ascontiguousarray(xb.T).astype(BF16)
        xqT_c = np.ascontiguousarray(xb[qc * QCHUNK : (qc + 1) * QCHUNK].T).astype(BF16)
        # exact blend constant: c = 0.5 * (sum_k v[k]) @ W_O^T
        sv = (xb.sum(0, dtype=np.float64) @ wvT.astype(np.float64))
        c = 0.5 * (sv @ W_O.T.astype(np.float64))
        in_maps.append(
            {
                "xkT": xkT_b,
                "xqT": xqT_c,
                "wqT": wqT.astype(BF16),
                "wkT": wkT.astype(BF16),
                "wvT": wvT.astype(BF16),
                "woT": woT.astype(BF16),
                "cvec": np.ascontiguousarray(c[None, :]).astype(np.float32),
            }
        )
    return in_maps


def kernel(x, W_K, W_Q, W_V, W_O):
    from concourse.bass_utils import run_bass_kernel_spmd

    nc = _get_program()
    in_maps = make_in_maps(x, W_K, W_Q, W_V, W_O)
    res = run_bass_kernel_spmd(nc, in_maps, core_ids=list(range(NCORES)))
    full = np.empty((B, S, D), np.float32)
    for core in range(NCORES):
        b, qc = divmod(core, 4)
        full[b, qc * QCHUNK : (qc + 1) * QCHUNK, :] = res.results[core]["out"]
    return full
